# revision 20
# baseline (speedup 1.0000x reference)
"""GQA attention kernel for Trainium2, sharded over 8 NeuronCores.

Problem: B=2, S=2048, D=2048, 32 query heads / 8 KV heads, head_dim=64.
Sharding: tensor-parallel over heads — core c owns query heads [4c, 4c+4)
and KV head c (exactly one GQA group), plus the matching slices of the
projections. Each core computes a partial of the output projection
(contraction over its 256 of the 2048 Wo rows); partials are summed on host.

On-device layout is fully "transposed": x arrives as x^T [D, B*S], the
projections produce q^T/k^T (token dim on the free axis), scores are computed
as S^T = K·q^T (so the softmax denominator is a partition-dim sum, obtained
for free from a ones-column appended to V), and the output projection emits
y^T = Wo^T · out^T. Softmax skips the max-subtraction (scores/8 ~ N(0,1); exp
cannot overflow). All matmuls run in bf16 with fp32 PSUM accumulation.

Schedule (v2): software-pipelined across batches — projections for batch 1
are emitted inside batch 0's attention, and each batch's output projection is
emitted inside its own m=1 attention pass. The two scores matmuls of a head
pair are adjacent 64x128 row-tiles (T0/T8) so they run concurrently on the PE
array; exp is one [128,1024] ACT instruction per key chunk covering both
heads. PSUM budget: sc 2x2 banks + av 2 + mm 2 = 8.
"""

import numpy as np
import ml_dtypes

import concourse.bacc as bacc
import concourse.mybir as mybir
import concourse.tile as tile
from concourse.bass_utils import run_bass_kernel_spmd
from concourse.masks import make_identity

BF16 = mybir.dt.bfloat16
F32 = mybir.dt.float32
AF = mybir.ActivationFunctionType
MULT = mybir.AluOpType.mult

B, S, D = 2, 2048, 2048
BS = B * S                    # 4096 tokens
NUM_HEADS, NUM_KV_HEADS, DH = 32, 8, 64
N_CORES = 8
HL = NUM_HEADS // N_CORES     # 4 local query heads
DQ = HL * DH                  # 256 local q dims
SC = S // 128                 # 16 key chunks per batch
SCALE = 1.0 / 8.0             # 1/sqrt(64)


def build_kernel(KT):
    """Build the per-core Bass program. KT = number of 128-row contraction
    tiles in the projections (16, or 17 when biases are folded in via an
    appended ones-row of x^T)."""
    nc = bacc.Bacc("TRN2", target_bir_lowering=False, debug=False,
                   num_devices=N_CORES)
    xt = nc.dram_tensor("xt", [KT * 128, BS], BF16, kind="ExternalInput").ap()
    wqkv = nc.dram_tensor("wqkv", [KT * 128, DQ + 2 * DH], BF16,
                          kind="ExternalInput").ap()
    wo = nc.dram_tensor("wo", [DQ, D], BF16, kind="ExternalInput").ap()
    yt = nc.dram_tensor("yt", [D, BS], BF16, kind="ExternalOutput").ap()

    xt_r = xt.rearrange("(kt p) n -> p kt n", p=128)
    wqkv_r = wqkv.rearrange("(kt p) m -> p kt m", p=128)
    wo_r = wo.rearrange("(kt p) n -> p kt n", p=128)

    with tile.TileContext(nc) as tc:
        with tc.tile_pool(name="singles", bufs=1) as singles, \
             tc.tile_pool(name="psum", bufs=1, space="PSUM") as psum, \
             tc.tile_pool(name="sbuf", bufs=1) as sbuf, \
             tc.tile_pool(name="xkpool", bufs=2) as xkpool, \
             tc.tile_pool(name="exppool", bufs=2) as exppool:

            # --- resident SBUF tensors -----------------------------------
            # wqkv arrives per m-tile so the first projection matmul only
            # waits for its own slice; the wo DMA is emitted much later (it
            # is first needed ~200us in, and would otherwise compete with
            # the critical first transfers).
            wqkv_sb = singles.tile([128, KT, DQ + 2 * DH], BF16, tag="wqkv")
            for m in range(3):
                msl = slice(m * 128, (m + 1) * 128)
                nc.sync.dma_start(wqkv_sb[:, :, msl], wqkv_r[:, :, msl])
            wo_sb = singles.tile([128, 2, D], BF16, tag="wo")

            # per-batch tensors (statically double-allocated)
            # q^T: m-tile 0 holds heads 0,1 (rows 0-63 / 64-127), m-tile 1
            # holds heads 2,3. k^T duplicated into both partition halves so
            # both row-tiles of the paired scores matmuls can read it.
            qt = [singles.tile([128, 2, S], BF16, tag=f"qt{b}",
                               name=f"qt{b}") for b in range(B)]
            kt2 = [singles.tile([128, S], BF16, tag=f"kt2{b}",
                                name=f"kt2{b}") for b in range(B)]
            # v in natural layout [token-chunk, 128]; columns 0-63 are all
            # 1.0 so the AV matmul lands 64 broadcast copies of the softmax
            # denominator on PSUM partitions 0-63 (base partition 0, which
            # the custom-DVE reciprocal requires); columns 64-127 hold the
            # 64 v dims, so the attention output lands on partitions 64-127.
            # Normalization then needs no PE broadcast, just a DVE
            # reciprocal + multiply.
            v_sb = [singles.tile([128, SC, 128], BF16, tag=f"v{b}",
                                 name=f"v{b}") for b in range(B)]
            ot = [singles.tile([128, 2, S], BF16, tag=f"ot{b}",
                               name=f"ot{b}") for b in range(B)]
            for b in range(B):
                for kt in range(SC):
                    nc.vector.memset(v_sb[b][:, kt, 0:DH], 1.0)

            ident = singles.tile([DH, DH], BF16, tag="ident")
            make_identity(nc, ident)

            # --- emission helpers ----------------------------------------
            xk_tiles = {}

            def emit_proj_part(b, blk, m):
                """One m-tile of the projections for one 512-token block.
                Parts are emitted in block-major order; the xk DMA is issued
                lazily (split in two so the first matmuls start sooner)."""
                gcols = slice(b * S + blk * 512, b * S + (blk + 1) * 512)
                cols = slice(blk * 512, (blk + 1) * 512)
                if (b, blk) not in xk_tiles:
                    xk = xkpool.tile([128, KT, 512], BF16, tag="xk")
                    half = KT // 2
                    nc.sync.dma_start(xk[:, 0:half, :],
                                      xt_r[:, 0:half, gcols])
                    nc.sync.dma_start(xk[:, half:, :],
                                      xt_r[:, half:, gcols])
                    xk_tiles[(b, blk)] = xk
                xk = xk_tiles[(b, blk)]
                pp = psum.tile([128, 512], F32, tag="mm", bufs=2)
                msl = slice(m * 128, (m + 1) * 128)
                for kt in range(KT):
                    nc.tensor.matmul(
                        pp[:], lhsT=wqkv_sb[:, kt, msl],
                        rhs=xk[:, kt, :],
                        start=(kt == 0), stop=(kt == KT - 1))
                if m < 2:
                    nc.vector.tensor_copy(out=qt[b][:, m, cols], in_=pp[:])
                else:
                    # rows 0-63: k^T (duplicate to both halves);
                    # rows 64-127: v^T (transpose into v_sb)
                    nc.vector.tensor_copy(out=kt2[b][0:64, cols],
                                          in_=pp[0:64, :])
                    nc.vector.tensor_copy(out=kt2[b][64:128, cols],
                                          in_=pp[0:64, :])
                    vt = sbuf.tile([64, 512], BF16, tag="vt", bufs=2)
                    nc.vector.tensor_copy(out=vt[:], in_=pp[64:128, :])
                    for j in range(4):
                        pt = psum.tile([128, DH], BF16, tag="mm",
                                       bufs=2)
                        nc.tensor.transpose(
                            pt[:], vt[:, j * 128:(j + 1) * 128],
                            ident[:])
                        nc.vector.tensor_copy(
                            out=v_sb[b][:, blk * 4 + j, DH:],
                            in_=pt[:])

            def emit_attn_unit(b, m, qb):
                """Attention for 512 queries of head-pair m, batch b."""
                qcols = slice(qb * 512, (qb + 1) * 512)
                exps = exppool.tile([128, SC, 2, 512], BF16, tag="exps")
                avs = [psum.tile([128, 512], F32, tag="av", bufs=2,
                                 name=f"av{h}") for h in range(2)]
                def emit_av(kt):
                    for h in range(2):
                        nc.tensor.matmul(
                            avs[h][:],
                            lhsT=v_sb[b][:, kt, :],
                            rhs=exps[:, kt, h, :],
                            start=(kt == 0), stop=(kt == SC - 1))

                # Process key chunks in pairs: both chunks' scores (64x128
                # row-tile mode), then the lagged AV matmuls (128x128 mode)
                # — halves the PE tiling-mode switches. AV lags one pair so
                # the PE FIFO never blocks on the exps it just requested.
                for kt0 in range(0, SC, 2):
                    for kt in (kt0, kt0 + 1):
                        kr = slice(kt * 128, (kt + 1) * 128)
                        # paired scores matmuls: h0 on rows 0-63 (tile T0),
                        # h1 on rows 64-127 (tile T8) — concurrent.
                        sc = psum.tile([128, 2, 512], F32, tag="sc", bufs=2)
                        for h in range(2):
                            rows = slice(h * 64, (h + 1) * 64)
                            nc.tensor.matmul(
                                sc[:, h, :], lhsT=kt2[b][rows, kr],
                                rhs=qt[b][rows, m, qcols],
                                start=True, stop=True)
                        # one exp for both heads: [128,1024] PSUM -> SBUF
                        nc.scalar.activation(
                            out=exps[:, kt, :, :], in_=sc[:, :, :],
                            func=AF.Exp, scale=SCALE)
                    if kt0 > 0:
                        emit_av(kt0 - 2)
                        emit_av(kt0 - 1)
                emit_av(SC - 2)
                emit_av(SC - 1)
                # normalization: 64 broadcast copies of the denominator sit
                # on partitions 0-63 of each AV accumulator, the attention
                # output on partitions 64-127.
                for h in range(2):
                    rows = slice(h * 64, (h + 1) * 64)
                    rec = sbuf.tile([64, 512], F32, tag="rec", bufs=2,
                                    name=f"rec{h}")
                    nc.vector.reciprocal_approx_fast(rec[:],
                                                     avs[h][0:64, :])
                    nc.vector.tensor_tensor(
                        out=ot[b][rows, m, qcols],
                        in0=avs[h][64:128, :], in1=rec[:],
                        op=MULT)

            def emit_oproj(b, qb):
                """Output projection for 512 queries of batch b (needs both
                head-pairs of this query block in ot)."""
                qcols = slice(qb * 512, (qb + 1) * 512)
                gcols = slice(b * S + qb * 512, b * S + (qb + 1) * 512)
                for dm in range(16):
                    dsl = slice(dm * 128, (dm + 1) * 128)
                    po = psum.tile([128, 512], F32, tag="mm", bufs=2)
                    for kt in range(2):
                        nc.tensor.matmul(
                            po[:], lhsT=wo_sb[:, kt, dsl],
                            rhs=ot[b][:, kt, qcols],
                            start=(kt == 0), stop=(kt == 1))
                    oc = sbuf.tile([128, 512], BF16, tag="oc", bufs=4)
                    nc.vector.tensor_copy(out=oc[:], in_=po[:])
                    nc.sync.dma_start(yt[dsl, gcols], oc[:])

            # --- pipelined schedule --------------------------------------
            # Batch-0 projections up front; batch-1 projection parts spread
            # evenly across all 8 batch-0 attention units; each batch's
            # output projection spread across the following 4 units so the
            # PE load per ACT-bound attention unit stays even.
            for blk in range(4):
                for m in range(3):
                    emit_proj_part(0, blk, m)
            nc.sync.dma_start(wo_sb[:], wo_r[:])
            b1_parts = [(blk, m) for blk in range(4) for m in range(3)]
            sched = [2, 1, 2, 1, 2, 1, 2, 1]  # parts per batch-0 unit
            for u, (mi, qb) in enumerate([(mi, qb) for mi in range(2)
                                          for qb in range(4)]):
                emit_attn_unit(0, mi, qb)
                for _ in range(sched[u]):
                    blk, m = b1_parts.pop(0)
                    emit_proj_part(1, blk, m)
            for qb in range(4):
                emit_attn_unit(1, 0, qb)
                emit_oproj(0, qb)
            for qb in range(4):
                emit_attn_unit(1, 1, qb)
                emit_oproj(1, qb)
    nc.compile()
    return nc


_CACHE = {}


def _get_kernel(KT):
    if KT not in _CACHE:
        _CACHE[KT] = build_kernel(KT)
    return _CACHE[KT]


def kernel(x, Wq, bq, Wk, bk, Wv, bv, Wo, bo):
    x = np.asarray(x, dtype=np.float32)
    Wq = np.asarray(Wq, dtype=np.float32)
    Wk = np.asarray(Wk, dtype=np.float32)
    Wv = np.asarray(Wv, dtype=np.float32)
    Wo = np.asarray(Wo, dtype=np.float32)
    bq = np.asarray(bq, dtype=np.float32)
    bk = np.asarray(bk, dtype=np.float32)
    bv = np.asarray(bv, dtype=np.float32)
    bo = np.asarray(bo, dtype=np.float32)
    assert x.shape == (B, S, D)

    has_bias = bool(np.any(bq) or np.any(bk) or np.any(bv))
    KT = 17 if has_bias else 16

    # x^T [D, B*S] (+ ones row when biases are folded into the projections)
    xt = np.zeros((KT * 128, BS), dtype=ml_dtypes.bfloat16)
    xt[:D] = x.transpose(2, 0, 1).reshape(D, BS).astype(ml_dtypes.bfloat16)
    if has_bias:
        xt[D] = 1.0

    nc = _get_kernel(KT)
    in_maps = []
    for c in range(N_CORES):
        qsl = slice(c * DQ, (c + 1) * DQ)
        ksl = slice(c * DH, (c + 1) * DH)
        wqkv = np.zeros((KT * 128, DQ + 2 * DH), dtype=ml_dtypes.bfloat16)
        wqkv[:D, :DQ] = Wq[:, qsl].astype(ml_dtypes.bfloat16)
        wqkv[:D, DQ:DQ + DH] = Wk[:, ksl].astype(ml_dtypes.bfloat16)
        wqkv[:D, DQ + DH:] = Wv[:, ksl].astype(ml_dtypes.bfloat16)
        if has_bias:
            wqkv[D, :DQ] = bq[qsl].astype(ml_dtypes.bfloat16)
            wqkv[D, DQ:DQ + DH] = bk[ksl].astype(ml_dtypes.bfloat16)
            wqkv[D, DQ + DH:] = bv[ksl].astype(ml_dtypes.bfloat16)
        in_maps.append({
            "xt": xt,
            "wqkv": wqkv,
            "wo": np.ascontiguousarray(Wo[qsl]).astype(ml_dtypes.bfloat16),
        })

    res = run_bass_kernel_spmd(nc, in_maps, core_ids=list(range(N_CORES)))
    yt = np.zeros((D, BS), dtype=np.float32)
    for r in res.results:
        yt += np.asarray(r["yt"], dtype=np.float32)
    y = yt.reshape(D, B, S).transpose(1, 2, 0) + bo
    return np.ascontiguousarray(y, dtype=np.float32)


# revision 24
# speedup vs baseline: 1.1706x; 1.1706x over previous
"""GQA attention kernel for Trainium2, sharded over 8 NeuronCores.

Problem: B=2, S=2048, D=2048, 32 query heads / 8 KV heads, head_dim=64.
Sharding: tensor-parallel over heads — core c owns query heads [4c, 4c+4)
and KV head c (exactly one GQA group), plus the matching slices of the
projections. Each core computes a partial of the output projection
(contraction over its 256 of the 2048 Wo rows); partials are summed on host.

On-device layout is fully "transposed": x arrives as x^T [D, B*S], the
projections produce q^T/k^T (token dim on the free axis), scores are computed
as S^T = K·q^T (so the softmax denominator is a partition-dim sum, obtained
for free from a ones-column appended to V), and the output projection emits
y^T = Wo^T · out^T. Softmax skips the max-subtraction (scores/8 ~ N(0,1); exp
cannot overflow). All matmuls run in bf16 with fp32 PSUM accumulation.

Schedule (v2): software-pipelined across batches — projections for batch 1
are emitted inside batch 0's attention, and each batch's output projection is
emitted inside its own m=1 attention pass. The two scores matmuls of a head
pair are adjacent 64x128 row-tiles (T0/T8) so they run concurrently on the PE
array; exp is one [128,1024] ACT instruction per key chunk covering both
heads. PSUM budget: sc 2x2 banks + av 2 + mm 2 = 8.
"""

import numpy as np
import ml_dtypes

import concourse.bacc as bacc
import concourse.mybir as mybir
import concourse.tile as tile
from concourse.bass_utils import run_bass_kernel_spmd
from concourse.masks import make_identity

BF16 = mybir.dt.bfloat16
F32 = mybir.dt.float32
AF = mybir.ActivationFunctionType
MULT = mybir.AluOpType.mult

B, S, D = 2, 2048, 2048
BS = B * S                    # 4096 tokens
NUM_HEADS, NUM_KV_HEADS, DH = 32, 8, 64
N_CORES = 8
HL = NUM_HEADS // N_CORES     # 4 local query heads
DQ = HL * DH                  # 256 local q dims
SC = S // 128                 # 16 key chunks per batch
SCALE = 1.0 / 8.0             # 1/sqrt(64)


def build_kernel(KT):
    """Build the per-core Bass program. KT = number of 128-row contraction
    tiles in the projections (16, or 17 when biases are folded in via an
    appended ones-row of x^T)."""
    nc = bacc.Bacc("TRN2", target_bir_lowering=False, debug=False,
                   num_devices=N_CORES)
    xt = nc.dram_tensor("xt", [KT * 128, BS], BF16, kind="ExternalInput").ap()
    wqkv = nc.dram_tensor("wqkv", [KT * 128, DQ + 2 * DH], BF16,
                          kind="ExternalInput").ap()
    wo = nc.dram_tensor("wo", [DQ, D], BF16, kind="ExternalInput").ap()
    yt = nc.dram_tensor("yt", [D, BS], BF16, kind="ExternalOutput").ap()

    xt_r = xt.rearrange("(kt p) n -> p kt n", p=128)
    wqkv_r = wqkv.rearrange("(kt p) m -> p kt m", p=128)
    wo_r = wo.rearrange("(kt p) n -> p kt n", p=128)

    with tile.TileContext(nc) as tc:
        with tc.tile_pool(name="singles", bufs=1) as singles, \
             tc.tile_pool(name="psum", bufs=1, space="PSUM") as psum, \
             tc.tile_pool(name="sbuf", bufs=1) as sbuf, \
             tc.tile_pool(name="xkpool", bufs=2) as xkpool, \
             tc.tile_pool(name="exppool", bufs=2) as exppool:

            # --- resident SBUF tensors -----------------------------------
            wqkv_sb = singles.tile([128, KT, DQ + 2 * DH], BF16, tag="wqkv")
            for m in range(3):
                msl = slice(m * 128, (m + 1) * 128)
                nc.sync.dma_start(wqkv_sb[:, :, msl], wqkv_r[:, :, msl])
            # wo is first needed ~200us in; its DMA is emitted after the
            # batch-0 projections so it cannot compete with the critical
            # first wqkv/xk transfers.
            wo_sb = singles.tile([128, 2, D], BF16, tag="wo")

            # per-batch tensors (statically double-allocated)
            # q^T: m-tile 0 holds heads 0,1 (rows 0-63 / 64-127), m-tile 1
            # holds heads 2,3. k^T duplicated into both partition halves so
            # both row-tiles of the paired scores matmuls can read it.
            qt = [singles.tile([128, 2, S], BF16, tag=f"qt{b}",
                               name=f"qt{b}") for b in range(B)]
            kt2 = [singles.tile([128, S], BF16, tag=f"kt2{b}",
                                name=f"kt2{b}") for b in range(B)]
            # v in natural layout [token-chunk, 128]; columns 0-63 are all
            # 1.0 so the AV matmul lands 64 broadcast copies of the softmax
            # denominator on PSUM partitions 0-63 (base partition 0, which
            # the custom-DVE reciprocal requires); columns 64-127 hold the
            # 64 v dims, so the attention output lands on partitions 64-127.
            # Normalization then needs no PE broadcast, just a DVE
            # reciprocal + multiply.
            v_sb = [singles.tile([128, SC, 128], BF16, tag=f"v{b}",
                                 name=f"v{b}") for b in range(B)]
            ot = [singles.tile([128, 2, S], BF16, tag=f"ot{b}",
                               name=f"ot{b}") for b in range(B)]
            for b in range(B):
                for kt in range(SC):
                    nc.vector.memset(v_sb[b][:, kt, 0:DH], 1.0)

            ident = singles.tile([DH, DH], BF16, tag="ident")
            make_identity(nc, ident)

            # --- emission helpers ----------------------------------------
            xk_tiles = {}

            def emit_proj_part(b, blk, m):
                """One m-tile of the projections for one 512-token block.
                Parts are emitted in block-major order; the xk DMA is issued
                lazily (split in two so the first matmuls start sooner)."""
                gcols = slice(b * S + blk * 512, b * S + (blk + 1) * 512)
                cols = slice(blk * 512, (blk + 1) * 512)
                if (b, blk) not in xk_tiles:
                    xk = xkpool.tile([128, KT, 512], BF16, tag="xk")
                    half = KT // 2
                    nc.sync.dma_start(xk[:, 0:half, :],
                                      xt_r[:, 0:half, gcols])
                    nc.sync.dma_start(xk[:, half:, :],
                                      xt_r[:, half:, gcols])
                    xk_tiles[(b, blk)] = xk
                xk = xk_tiles[(b, blk)]
                pp = psum.tile([128, 512], F32, tag="mm", bufs=2)
                msl = slice(m * 128, (m + 1) * 128)
                for kt in range(KT):
                    nc.tensor.matmul(
                        pp[:], lhsT=wqkv_sb[:, kt, msl],
                        rhs=xk[:, kt, :],
                        start=(kt == 0), stop=(kt == KT - 1))
                if m < 2:
                    nc.vector.tensor_copy(out=qt[b][:, m, cols], in_=pp[:])
                else:
                    # rows 0-63: k^T (duplicate to both halves);
                    # rows 64-127: v^T (transpose into v_sb)
                    nc.vector.tensor_copy(out=kt2[b][0:64, cols],
                                          in_=pp[0:64, :])
                    nc.vector.tensor_copy(out=kt2[b][64:128, cols],
                                          in_=pp[0:64, :])
                    vt = sbuf.tile([64, 512], BF16, tag="vt", bufs=2)
                    nc.vector.tensor_copy(out=vt[:], in_=pp[64:128, :])
                    for j in range(4):
                        pt = psum.tile([128, DH], BF16, tag="mm",
                                       bufs=2)
                        nc.tensor.transpose(
                            pt[:], vt[:, j * 128:(j + 1) * 128],
                            ident[:])
                        nc.vector.tensor_copy(
                            out=v_sb[b][:, blk * 4 + j, DH:],
                            in_=pt[:])

            def emit_attn_unit(b, m, qb):
                """Attention for 512 queries of head-pair m, batch b."""
                qcols = slice(qb * 512, (qb + 1) * 512)
                exps = exppool.tile([128, SC, 2, 512], BF16, tag="exps")
                avs = [psum.tile([128, 512], F32, tag="av", bufs=2,
                                 name=f"av{h}") for h in range(2)]
                def emit_av(kt):
                    for h in range(2):
                        nc.tensor.matmul(
                            avs[h][:],
                            lhsT=v_sb[b][:, kt, :],
                            rhs=exps[:, kt, h, :],
                            start=(kt == 0), stop=(kt == SC - 1))

                # Process key chunks in pairs: both chunks' scores (64x128
                # row-tile mode), then the lagged AV matmuls (128x128 mode)
                # — halves the PE tiling-mode switches. AV lags one pair so
                # the PE FIFO never blocks on the exps it just requested.
                for kt0 in range(0, SC, 2):
                    for kt in (kt0, kt0 + 1):
                        kr = slice(kt * 128, (kt + 1) * 128)
                        # paired scores matmuls: h0 on rows 0-63 (tile T0),
                        # h1 on rows 64-127 (tile T8) — concurrent.
                        sc = psum.tile([128, 2, 512], F32, tag="sc", bufs=2)
                        for h in range(2):
                            rows = slice(h * 64, (h + 1) * 64)
                            nc.tensor.matmul(
                                sc[:, h, :], lhsT=kt2[b][rows, kr],
                                rhs=qt[b][rows, m, qcols],
                                start=True, stop=True)
                        # one exp for both heads: [128,1024] PSUM -> SBUF
                        nc.scalar.activation(
                            out=exps[:, kt, :, :], in_=sc[:, :, :],
                            func=AF.Exp, scale=SCALE)
                    if kt0 > 0:
                        emit_av(kt0 - 2)
                        emit_av(kt0 - 1)
                emit_av(SC - 2)
                emit_av(SC - 1)
                # normalization: 64 broadcast copies of the denominator sit
                # on partitions 0-63 of each AV accumulator, the attention
                # output on partitions 64-127.
                for h in range(2):
                    rows = slice(h * 64, (h + 1) * 64)
                    rec = sbuf.tile([64, 512], F32, tag="rec", bufs=2,
                                    name=f"rec{h}")
                    nc.vector.reciprocal_approx_fast(rec[:],
                                                     avs[h][0:64, :])
                    nc.vector.tensor_tensor(
                        out=ot[b][rows, m, qcols],
                        in0=avs[h][64:128, :], in1=rec[:],
                        op=MULT)

            def emit_oproj(b, qb, dm_lo=0, dm_hi=16):
                """Output projection for 512 queries of batch b (needs both
                head-pairs of this query block in ot)."""
                qcols = slice(qb * 512, (qb + 1) * 512)
                gcols = slice(b * S + qb * 512, b * S + (qb + 1) * 512)
                for dm in range(dm_lo, dm_hi):
                    dsl = slice(dm * 128, (dm + 1) * 128)
                    po = psum.tile([128, 512], F32, tag="mm", bufs=2)
                    for kt in range(2):
                        nc.tensor.matmul(
                            po[:], lhsT=wo_sb[:, kt, dsl],
                            rhs=ot[b][:, kt, qcols],
                            start=(kt == 0), stop=(kt == 1))
                    oc = sbuf.tile([128, 512], BF16, tag="oc", bufs=4)
                    nc.vector.tensor_copy(out=oc[:], in_=po[:])
                    nc.sync.dma_start(yt[dsl, gcols], oc[:])

            # --- pipelined schedule --------------------------------------
            # Batch-0 projections up front; batch-1 projection parts spread
            # across the batch-0 attention units; each batch's output
            # projection spread across all units where its ot data is ready
            # (5 tiles right after a query block's m=1 unit, the remaining
            # 11 one round later) so the PE load per ACT-bound attention
            # unit stays even and the po-bank rotation keeps slack.
            for blk in range(4):
                for m in range(3):
                    emit_proj_part(0, blk, m)
            nc.sync.dma_start(wo_sb[:], wo_r[:])
            b1_parts = [(blk, m) for blk in range(4) for m in range(3)]
            for qb in range(4):
                emit_attn_unit(0, 0, qb)
                for _ in range(2):
                    blk, m = b1_parts.pop(0)
                    emit_proj_part(1, blk, m)
            for qb in range(4):
                emit_attn_unit(0, 1, qb)
                emit_oproj(0, qb, 0, 5)
                blk, m = b1_parts.pop(0)
                emit_proj_part(1, blk, m)
            for qb in range(4):
                emit_attn_unit(1, 0, qb)
                emit_oproj(0, qb, 5, 16)
            for qb in range(4):
                emit_attn_unit(1, 1, qb)
                emit_oproj(1, qb)
    nc.compile()
    return nc


_CACHE = {}


def _get_kernel(KT):
    if KT not in _CACHE:
        _CACHE[KT] = build_kernel(KT)
    return _CACHE[KT]


def kernel(x, Wq, bq, Wk, bk, Wv, bv, Wo, bo):
    x = np.asarray(x, dtype=np.float32)
    Wq = np.asarray(Wq, dtype=np.float32)
    Wk = np.asarray(Wk, dtype=np.float32)
    Wv = np.asarray(Wv, dtype=np.float32)
    Wo = np.asarray(Wo, dtype=np.float32)
    bq = np.asarray(bq, dtype=np.float32)
    bk = np.asarray(bk, dtype=np.float32)
    bv = np.asarray(bv, dtype=np.float32)
    bo = np.asarray(bo, dtype=np.float32)
    assert x.shape == (B, S, D)

    has_bias = bool(np.any(bq) or np.any(bk) or np.any(bv))
    KT = 17 if has_bias else 16

    # x^T [D, B*S] (+ ones row when biases are folded into the projections)
    xt = np.zeros((KT * 128, BS), dtype=ml_dtypes.bfloat16)
    xt[:D] = x.transpose(2, 0, 1).reshape(D, BS).astype(ml_dtypes.bfloat16)
    if has_bias:
        xt[D] = 1.0

    nc = _get_kernel(KT)
    in_maps = []
    for c in range(N_CORES):
        qsl = slice(c * DQ, (c + 1) * DQ)
        ksl = slice(c * DH, (c + 1) * DH)
        wqkv = np.zeros((KT * 128, DQ + 2 * DH), dtype=ml_dtypes.bfloat16)
        wqkv[:D, :DQ] = Wq[:, qsl].astype(ml_dtypes.bfloat16)
        wqkv[:D, DQ:DQ + DH] = Wk[:, ksl].astype(ml_dtypes.bfloat16)
        wqkv[:D, DQ + DH:] = Wv[:, ksl].astype(ml_dtypes.bfloat16)
        if has_bias:
            wqkv[D, :DQ] = bq[qsl].astype(ml_dtypes.bfloat16)
            wqkv[D, DQ:DQ + DH] = bk[ksl].astype(ml_dtypes.bfloat16)
            wqkv[D, DQ + DH:] = bv[ksl].astype(ml_dtypes.bfloat16)
        in_maps.append({
            "xt": xt,
            "wqkv": wqkv,
            "wo": np.ascontiguousarray(Wo[qsl]).astype(ml_dtypes.bfloat16),
        })

    res = run_bass_kernel_spmd(nc, in_maps, core_ids=list(range(N_CORES)))
    yt = np.zeros((D, BS), dtype=np.float32)
    for r in res.results:
        yt += np.asarray(r["yt"], dtype=np.float32)
    y = yt.reshape(D, B, S).transpose(1, 2, 0) + bo
    return np.ascontiguousarray(y, dtype=np.float32)
